# revision 8
# baseline (speedup 1.0000x reference)
"""Trainium2 Bass kernel for Llama4TextExperts (MoE expert MLP chain).

Problem: E=8 experts, T=2048 tokens/expert, H=2048 hidden, D=4096 intermediate.
  hs (E*T, H) -> per expert e: g = hs_e @ Wg_e; u = hs_e @ Wu_e;
  f = u * silu(g); y_e = f @ Wd_e  -> out (E*T, H), all fp32.

Sharding: expert-parallel, 1 expert per NeuronCore (8 cores).

Per-core kernel design (v4):
  - All matmul operands bf16 (full PE rate + FWL on every LDWEIGHTS).
  - Host PACKS every streamed tensor so each DMA block is one fully
    contiguous DRAM region with 8-16KB contiguous per-partition runs
    (sub-1KB lines were demonstrably descriptor-rate-bound):
      xp  [N_TT, 128, N_H, TT]    xp[t,p,n,:]   = x[n*128+p, t*TT:(t+1)*TT]
      wgp [NB, 128, N_H, WGD]     wgp[b,p,n,:]  = wg[n*128+p, b*WGD:(b+1)*WGD]
      wdp [N_HC, N_DG, 128, 8, HC] wdp[c,g,p,k,:] = wd[(g*8+k)*128+p, c*HC:...]
  - ~24 dummy warmup matmuls at program start get the PE clock gate to
    K=8/8 and bridge the initial x/weight DMA latency.
  - Loop over T in tiles of TT=512 tokens:
      stage 1: per 128-wide d-tile: psum_g/psum_u accumulate 16 matmuls
        over h-chunks; silu on ScalarE; f = silu(g)*u on VectorE -> bf16
        f_T[d] tiles [128(d) x 512(t)].
      stage 2 (ts-outer): per 512-wide output h-chunk: 4 wd d-group tiles,
        then per 128-token subtile: psum_y accumulates 32 matmuls -> copy
        -> DMA out. Early psum drains keep the tail short.
  - x loads + y stores ride the qAct HWDGE ring (nc.scalar.dma_start),
    weight streams ride qSP (nc.sync) - independent queues.
"""

import os
import sys

for _p in ("/opt/trn_rl_repo",):
    if _p not in sys.path and os.path.isdir(_p):
        sys.path.insert(0, _p)

import numpy as np
from ml_dtypes import bfloat16 as bf16

E = 8
T = 2048
H = 2048
D = 4096

P = 128
TT = 512
WGD = 256
HC = 512
WD_DCH = 8
N_H = H // P
N_D = D // P
N_TT = T // TT
TS = TT // P
N_HC = H // HC
N_DG = N_D // WD_DCH
NB = D // WGD

_CACHE = {}


def _build_bass():
    """Build the single-core Bass module (same program for all 8 cores)."""
    import concourse.bass as bass
    import concourse.mybir as mybir
    from concourse.tile import TileContext

    f32 = mybir.dt.float32
    bf = mybir.dt.bfloat16

    nc = bass.Bass(trn_type="TRN2")

    xp = nc.declare_dram_parameter("xp", [N_TT, P, N_H, TT], bf, isOutput=False)
    wgp = nc.declare_dram_parameter("wgp", [NB, P, N_H, WGD], bf, isOutput=False)
    wup = nc.declare_dram_parameter("wup", [NB, P, N_H, WGD], bf, isOutput=False)
    wdp = nc.declare_dram_parameter(
        "wdp", [N_HC, N_DG, P, WD_DCH, HC], bf, isOutput=False)
    y = nc.declare_dram_parameter("y", [T, H], f32, isOutput=True)

    y_r = y[:].rearrange("(n p) h -> p n h", p=P)      # [128, T//128, H]

    with TileContext(nc) as tc:
        with (
            tc.tile_pool(name="xpool", bufs=2) as xpool,
            tc.tile_pool(name="wpool", bufs=2) as wpool,
            tc.tile_pool(name="dpool", bufs=1) as dpool,
            tc.tile_pool(name="wdpool", bufs=2) as wdpool,
            tc.tile_pool(name="fpool", bufs=N_D) as fpool,
            tc.tile_pool(name="spool", bufs=6) as spool,
            tc.tile_pool(name="ypool", bufs=4) as ypool,
            tc.tile_pool(name="pgu", bufs=2, space="PSUM") as pgu,
            tc.tile_pool(name="py", bufs=4, space="PSUM") as py,
        ):
            # ---- HAM warmup: dummy matmuls keep the PE busy (and get the
            # clock gate to K=8/8) while the first x/weight DMAs stream in.
            # N=256 keeps the granularity fine so little time is wasted when
            # the real operands land.
            N_WARM = 88
            DN = 256
            dum_w = dpool.tile([P, P], bf, tag="dw")
            dum_x = dpool.tile([P, DN], bf, tag="dx")
            nc.vector.memset(dum_w[:], 0.0)
            nc.vector.memset(dum_x[:], 0.0)
            dum_ps = py.tile([P, HC], f32, tag="py")
            for _ in range(N_WARM):
                nc.tensor.matmul(dum_ps[:, 0:DN], lhsT=dum_w, rhs=dum_x,
                                 start=True, stop=True)

            def load_x(tt):
                x_t = xpool.tile([P, N_H, TT], bf, tag="x")
                nc.scalar.dma_start(out=x_t, in_=xp[tt])
                return x_t

            x_cur = load_x(0)
            for tt in range(N_TT):
                # ---- stage 1: gate/up + swiglu, d-tile at a time
                f_tiles = []
                for b in range(NB):
                    wg_t = wpool.tile([P, N_H, WGD], bf, tag="wg")
                    wu_t = wpool.tile([P, N_H, WGD], bf, tag="wu")
                    if tt == 0 and b == 0:
                        # split the very first loads so the first matmuls can
                        # start after half a block has landed
                        hh = N_H // 2
                        nc.sync.dma_start(out=wg_t[:, 0:hh, :],
                                          in_=wgp[b, :, 0:hh, :])
                        nc.sync.dma_start(out=wu_t[:, 0:hh, :],
                                          in_=wup[b, :, 0:hh, :])
                        nc.sync.dma_start(out=wg_t[:, hh:N_H, :],
                                          in_=wgp[b, :, hh:N_H, :])
                        nc.sync.dma_start(out=wu_t[:, hh:N_H, :],
                                          in_=wup[b, :, hh:N_H, :])
                    else:
                        nc.sync.dma_start(out=wg_t, in_=wgp[b])
                        nc.sync.dma_start(out=wu_t, in_=wup[b])
                    for dw in range(WGD // P):
                        psum_g = pgu.tile([P, TT], f32, tag="pg")
                        psum_u = pgu.tile([P, TT], f32, tag="pu")
                        for h in range(N_H):
                            nc.tensor.matmul(
                                psum_g,
                                lhsT=wg_t[:, h, dw * P:(dw + 1) * P],
                                rhs=x_cur[:, h, :],
                                start=(h == 0), stop=(h == N_H - 1),
                            )
                        for h in range(N_H):
                            nc.tensor.matmul(
                                psum_u,
                                lhsT=wu_t[:, h, dw * P:(dw + 1) * P],
                                rhs=x_cur[:, h, :],
                                start=(h == 0), stop=(h == N_H - 1),
                            )
                        s_t = spool.tile([P, TT], f32, tag="s")
                        nc.scalar.activation(
                            out=s_t, in_=psum_g,
                            func=mybir.ActivationFunctionType.Silu,
                        )
                        f_t = fpool.tile([P, TT], bf, tag="f")
                        nc.vector.tensor_mul(f_t, s_t, psum_u)
                        f_tiles.append(f_t)

                # ---- prefetch next t-tile's x ahead of stage-2 y stores
                x_next = load_x(tt + 1) if tt + 1 < N_TT else None

                # ---- stage 2: y[t, h] = sum_d f_T[d, t] * wd[d, h]
                # ts-outer: each psum bank finishes all 32 d accumulations,
                # then drains (copy + DMA) while the next ts is computing.
                for hc in range(N_HC):
                    hsl = slice(hc * HC, (hc + 1) * HC)
                    wd_ts = []
                    for dg in range(N_DG):
                        wd_t = wdpool.tile([P, WD_DCH, HC], bf, tag=f"wd{dg}")
                        nc.sync.dma_start(out=wd_t, in_=wdp[hc, dg])
                        wd_ts.append(wd_t)
                    for ts in range(TS):
                        psum_y = py.tile([P, HC], f32, tag="py")
                        for dg in range(N_DG):
                            for dc in range(WD_DCH):
                                dt = dg * WD_DCH + dc
                                nc.tensor.matmul(
                                    psum_y,
                                    lhsT=f_tiles[dt][:, ts * P:(ts + 1) * P],
                                    rhs=wd_ts[dg][:, dc, :],
                                    start=(dt == 0), stop=(dt == N_D - 1),
                                )
                        y_sb = ypool.tile([P, HC], f32, tag="y")
                        nc.scalar.copy(out=y_sb, in_=psum_y)
                        nc.scalar.dma_start(
                            out=y_r[:, tt * TS + ts, hsl], in_=y_sb,
                        )
                x_cur = x_next
    _split_matmul_waits(nc)
    return nc


def _split_matmul_waits(nc):
    """walrus splits Matmult into LDW+MM and moves the Matmult's sync
    waits onto the generated LW struct, which has room for only one wait.
    Hoist every Matmult's waits onto a PE InstNoOp inserted just before it."""
    import concourse.mybir as mybir

    for f in nc.m.functions:
        for bb in f.blocks:
            insts = list(bb.instructions)
            out = []
            n_nops = 0
            for ins in insts:
                si = ins.sync_info
                tname = type(ins).__name__
                if (
                    si is not None
                    and len(si.on_wait) > (1 if tname != "InstMatmult" else 0)
                ):
                    keep = [] if tname == "InstMatmult" else [si.on_wait[-1]]
                    hoist = si.on_wait if tname == "InstMatmult" else si.on_wait[:-1]
                    for i, w in enumerate(hoist):
                        nop = mybir.InstNoOp(
                            name=f"{ins.name}-waitnop{i}",
                            engine=ins.engine,
                            ins=[],
                            outs=[],
                            sync_info=mybir.SyncInfo(
                                on_wait=[w], on_update=[]
                            ),
                        )
                        out.append(nop)
                        n_nops += 1
                    ins.sync_info = mybir.SyncInfo(
                        on_wait=keep, on_update=list(si.on_update)
                    )
                out.append(ins)
            if n_nops:
                bb.instructions = out


def make_in_maps(hidden_states, gate_proj, up_proj, down_proj):
    hs = np.ascontiguousarray(hidden_states, dtype=np.float32).reshape(E, T, H)
    in_maps = []
    for e in range(E):
        # xp[t, p, n, :] = x[tok, h=n*128+p] transposed -> xT[h, tok-slice]
        xT = hs[e].T.astype(bf16)                       # [H, T]
        xp = np.ascontiguousarray(
            xT.reshape(N_H, P, N_TT, TT).transpose(2, 1, 0, 3))
        # wgp[b, p, n, :] = wg[n*128+p, b*WGD:(b+1)*WGD]
        wg = np.asarray(gate_proj[e], dtype=np.float32).astype(bf16)
        wgp = np.ascontiguousarray(
            wg.reshape(N_H, P, NB, WGD).transpose(2, 1, 0, 3))
        wu = np.asarray(up_proj[e], dtype=np.float32).astype(bf16)
        wup = np.ascontiguousarray(
            wu.reshape(N_H, P, NB, WGD).transpose(2, 1, 0, 3))
        # wdp[c, g, p, k, :] = wd[(g*8+k)*128+p, c*HC:(c+1)*HC]
        wd = np.asarray(down_proj[e], dtype=np.float32).astype(bf16)
        wdp = np.ascontiguousarray(
            wd.reshape(N_DG, WD_DCH, P, N_HC, HC).transpose(3, 0, 2, 1, 4))
        in_maps.append({"xp": xp, "wgp": wgp, "wup": wup, "wdp": wdp})
    return in_maps


def kernel(hidden_states, gate_proj, up_proj, down_proj):
    from concourse.bass_utils import run_bass_kernel_spmd

    in_maps = make_in_maps(hidden_states, gate_proj, up_proj, down_proj)
    if "nc" not in _CACHE:
        _CACHE["nc"] = _build_bass()
    nc = _CACHE["nc"]

    res = run_bass_kernel_spmd(nc, in_maps, core_ids=list(range(E)))
    out = np.concatenate([res.results[e]["y"] for e in range(E)], axis=0)
    return out.astype(np.float32)


if __name__ == "__main__":
    # smoke: build only
    nc = _build_bass()
    print("built ok, instructions:", len(nc.inst_map))


# revision 13
# speedup vs baseline: 1.0025x; 1.0025x over previous
"""Trainium2 Bass kernel for Llama4TextExperts (MoE expert MLP chain).

Problem: E=8 experts, T=2048 tokens/expert, H=2048 hidden, D=4096 intermediate.
  hs (E*T, H) -> per expert e: g = hs_e @ Wg_e; u = hs_e @ Wu_e;
  f = u * silu(g); y_e = f @ Wd_e  -> out (E*T, H), all fp32.

Sharding: expert-parallel, 1 expert per NeuronCore (8 cores).

Per-core kernel design (v4):
  - All matmul operands bf16 (full PE rate + FWL on every LDWEIGHTS).
  - Host PACKS every streamed tensor so each DMA block is one fully
    contiguous DRAM region with 8-16KB contiguous per-partition runs
    (sub-1KB lines were demonstrably descriptor-rate-bound):
      xp  [N_TT, 128, N_H, TT]    xp[t,p,n,:]   = x[n*128+p, t*TT:(t+1)*TT]
      wgp [NB, 128, N_H, WGD]     wgp[b,p,n,:]  = wg[n*128+p, b*WGD:(b+1)*WGD]
      wdp [N_HC, N_DG, 128, 8, HC] wdp[c,g,p,k,:] = wd[(g*8+k)*128+p, c*HC:...]
  - ~24 dummy warmup matmuls at program start get the PE clock gate to
    K=8/8 and bridge the initial x/weight DMA latency.
  - Loop over T in tiles of TT=512 tokens:
      stage 1: per 128-wide d-tile: psum_g/psum_u accumulate 16 matmuls
        over h-chunks; silu on ScalarE; f = silu(g)*u on VectorE -> bf16
        f_T[d] tiles [128(d) x 512(t)].
      stage 2 (ts-outer): per 512-wide output h-chunk: 4 wd d-group tiles,
        then per 128-token subtile: psum_y accumulates 32 matmuls -> copy
        -> DMA out. Early psum drains keep the tail short.
  - x loads + y stores ride the qAct HWDGE ring (nc.scalar.dma_start),
    weight streams ride qSP (nc.sync) - independent queues.
"""

import os
import sys

for _p in ("/opt/trn_rl_repo",):
    if _p not in sys.path and os.path.isdir(_p):
        sys.path.insert(0, _p)

import numpy as np
from ml_dtypes import bfloat16 as bf16

E = 8
T = 2048
H = 2048
D = 4096

P = 128
TT = 512
WGD = 256
HC = 512
WD_DCH = 8
N_H = H // P
N_D = D // P
N_TT = T // TT
TS = TT // P
N_HC = H // HC
N_DG = N_D // WD_DCH
NB = D // WGD

_CACHE = {}


def _build_bass():
    """Build the single-core Bass module (same program for all 8 cores)."""
    import concourse.bass as bass
    import concourse.mybir as mybir
    from concourse.tile import TileContext

    f32 = mybir.dt.float32
    bf = mybir.dt.bfloat16

    nc = bass.Bass(trn_type="TRN2")

    xp = nc.declare_dram_parameter("xp", [N_TT, P, N_H, TT], bf, isOutput=False)
    wgp = nc.declare_dram_parameter("wgp", [NB, P, N_H, WGD], bf, isOutput=False)
    wup = nc.declare_dram_parameter("wup", [NB, P, N_H, WGD], bf, isOutput=False)
    wdp = nc.declare_dram_parameter(
        "wdp", [N_HC, N_DG, P, WD_DCH, HC], bf, isOutput=False)
    y = nc.declare_dram_parameter("y", [T, H], f32, isOutput=True)

    y_r = y[:].rearrange("(n p) h -> p n h", p=P)      # [128, T//128, H]

    with TileContext(nc) as tc:
        with (
            tc.tile_pool(name="xpool", bufs=2) as xpool,
            tc.tile_pool(name="wpool", bufs=2) as wpool,
            tc.tile_pool(name="dpool", bufs=1) as dpool,
            tc.tile_pool(name="wdpool", bufs=2) as wdpool,
            tc.tile_pool(name="fpool", bufs=N_D) as fpool,
            tc.tile_pool(name="spool", bufs=6) as spool,
            tc.tile_pool(name="ypool", bufs=4) as ypool,
            tc.tile_pool(name="pgu", bufs=2, space="PSUM") as pgu,
            tc.tile_pool(name="py", bufs=4, space="PSUM") as py,
        ):
            # ---- HAM warmup: dummy matmuls keep the PE busy (and get the
            # clock gate to K=8/8) while the first x/weight DMAs stream in.
            # N=256 keeps the granularity fine so little time is wasted when
            # the real operands land.
            N_WARM = 40
            DN = 256
            dum_w = dpool.tile([P, P], bf, tag="dw")
            dum_x = dpool.tile([P, DN], bf, tag="dx")
            nc.vector.memset(dum_w[:], 0.0)
            nc.vector.memset(dum_x[:], 0.0)
            dum_ps = py.tile([P, HC], f32, tag="py")
            for _ in range(N_WARM):
                nc.tensor.matmul(dum_ps[:, 0:DN], lhsT=dum_w, rhs=dum_x,
                                 start=True, stop=True)

            def load_x(tt, split=False):
                x_t = xpool.tile([P, N_H, TT], bf, tag="x")
                if split:
                    # halves on both HWDGE rings for 2x startup bandwidth
                    hh = N_H // 2
                    nc.scalar.dma_start(out=x_t[:, 0:hh, :],
                                        in_=xp[tt, :, 0:hh, :])
                    nc.sync.dma_start(out=x_t[:, hh:N_H, :],
                                      in_=xp[tt, :, hh:N_H, :])
                else:
                    nc.scalar.dma_start(out=x_t, in_=xp[tt])
                return x_t

            def load_wgu(b):
                # wg rides qSP, wu rides qAct - parallel weight streams
                wg_t = wpool.tile([P, N_H, WGD], bf, tag="wg")
                wu_t = wpool.tile([P, N_H, WGD], bf, tag="wu")
                nc.sync.dma_start(out=wg_t, in_=wgp[b])
                nc.scalar.dma_start(out=wu_t, in_=wup[b])
                return wg_t, wu_t

            x_cur = load_x(0, split=True)
            b0_pre = None
            for tt in range(N_TT):
                # ---- stage 1: gate/up + swiglu, d-tile at a time
                f_tiles = []
                for b in range(NB):
                    if b == 0 and b0_pre is not None:
                        wg_t, wu_t = b0_pre
                    else:
                        wg_t, wu_t = load_wgu(b)
                    for dw in range(WGD // P):
                        psum_g = pgu.tile([P, TT], f32, tag="pg")
                        psum_u = pgu.tile([P, TT], f32, tag="pu")
                        for h in range(N_H):
                            nc.tensor.matmul(
                                psum_g,
                                lhsT=wg_t[:, h, dw * P:(dw + 1) * P],
                                rhs=x_cur[:, h, :],
                                start=(h == 0), stop=(h == N_H - 1),
                            )
                        for h in range(N_H):
                            nc.tensor.matmul(
                                psum_u,
                                lhsT=wu_t[:, h, dw * P:(dw + 1) * P],
                                rhs=x_cur[:, h, :],
                                start=(h == 0), stop=(h == N_H - 1),
                            )
                        s_t = spool.tile([P, TT], f32, tag="s")
                        nc.scalar.activation(
                            out=s_t, in_=psum_g,
                            func=mybir.ActivationFunctionType.Silu,
                        )
                        f_t = fpool.tile([P, TT], bf, tag="f")
                        nc.vector.tensor_mul(f_t, s_t, psum_u)
                        f_tiles.append(f_t)

                # ---- prefetch next t-tile's x ahead of stage-2 y stores
                x_next = load_x(tt + 1) if tt + 1 < N_TT else None
                b0_pre = None

                # ---- stage 2: y[t, h] = sum_d f_T[d, t] * wd[d, h]
                # ts-outer: each psum bank finishes all 32 d accumulations,
                # then drains (copy + DMA) while the next ts is computing.
                for hc in range(N_HC):
                    hsl = slice(hc * HC, (hc + 1) * HC)
                    wd_ts = []
                    for dg in range(N_DG):
                        wd_t = wdpool.tile([P, WD_DCH, HC], bf, tag=f"wd{dg}")
                        nc.sync.dma_start(out=wd_t, in_=wdp[hc, dg])
                        wd_ts.append(wd_t)
                    if hc == 1 and tt + 1 < N_TT:
                        # prefetch next t-tile's first weight block behind
                        # wd-hc0/hc1 in queue order
                        b0_pre = load_wgu(0)
                    for ts in range(TS):
                        psum_y = py.tile([P, HC], f32, tag="py")
                        for dg in range(N_DG):
                            for dc in range(WD_DCH):
                                dt = dg * WD_DCH + dc
                                nc.tensor.matmul(
                                    psum_y,
                                    lhsT=f_tiles[dt][:, ts * P:(ts + 1) * P],
                                    rhs=wd_ts[dg][:, dc, :],
                                    start=(dt == 0), stop=(dt == N_D - 1),
                                )
                        y_sb = ypool.tile([P, HC], f32, tag="y")
                        nc.scalar.copy(out=y_sb, in_=psum_y)
                        nc.scalar.dma_start(
                            out=y_r[:, tt * TS + ts, hsl], in_=y_sb,
                        )
                x_cur = x_next
    _split_matmul_waits(nc)
    return nc


def _split_matmul_waits(nc):
    """walrus splits Matmult into LDW+MM and moves the Matmult's sync
    waits onto the generated LW struct, which has room for only one wait.
    Hoist every Matmult's waits onto a PE InstNoOp inserted just before it."""
    import concourse.mybir as mybir

    for f in nc.m.functions:
        for bb in f.blocks:
            insts = list(bb.instructions)
            out = []
            n_nops = 0
            for ins in insts:
                si = ins.sync_info
                tname = type(ins).__name__
                if (
                    si is not None
                    and len(si.on_wait) > (1 if tname != "InstMatmult" else 0)
                ):
                    keep = [] if tname == "InstMatmult" else [si.on_wait[-1]]
                    hoist = si.on_wait if tname == "InstMatmult" else si.on_wait[:-1]
                    for i, w in enumerate(hoist):
                        nop = mybir.InstNoOp(
                            name=f"{ins.name}-waitnop{i}",
                            engine=ins.engine,
                            ins=[],
                            outs=[],
                            sync_info=mybir.SyncInfo(
                                on_wait=[w], on_update=[]
                            ),
                        )
                        out.append(nop)
                        n_nops += 1
                    ins.sync_info = mybir.SyncInfo(
                        on_wait=keep, on_update=list(si.on_update)
                    )
                out.append(ins)
            if n_nops:
                bb.instructions = out


def make_in_maps(hidden_states, gate_proj, up_proj, down_proj):
    hs = np.ascontiguousarray(hidden_states, dtype=np.float32).reshape(E, T, H)
    in_maps = []
    for e in range(E):
        # xp[t, p, n, :] = x[tok, h=n*128+p] transposed -> xT[h, tok-slice]
        xT = hs[e].T.astype(bf16)                       # [H, T]
        xp = np.ascontiguousarray(
            xT.reshape(N_H, P, N_TT, TT).transpose(2, 1, 0, 3))
        # wgp[b, p, n, :] = wg[n*128+p, b*WGD:(b+1)*WGD]
        wg = np.asarray(gate_proj[e], dtype=np.float32).astype(bf16)
        wgp = np.ascontiguousarray(
            wg.reshape(N_H, P, NB, WGD).transpose(2, 1, 0, 3))
        wu = np.asarray(up_proj[e], dtype=np.float32).astype(bf16)
        wup = np.ascontiguousarray(
            wu.reshape(N_H, P, NB, WGD).transpose(2, 1, 0, 3))
        # wdp[c, g, p, k, :] = wd[(g*8+k)*128+p, c*HC:(c+1)*HC]
        wd = np.asarray(down_proj[e], dtype=np.float32).astype(bf16)
        wdp = np.ascontiguousarray(
            wd.reshape(N_DG, WD_DCH, P, N_HC, HC).transpose(3, 0, 2, 1, 4))
        in_maps.append({"xp": xp, "wgp": wgp, "wup": wup, "wdp": wdp})
    return in_maps


def kernel(hidden_states, gate_proj, up_proj, down_proj):
    from concourse.bass_utils import run_bass_kernel_spmd

    in_maps = make_in_maps(hidden_states, gate_proj, up_proj, down_proj)
    if "nc" not in _CACHE:
        _CACHE["nc"] = _build_bass()
    nc = _CACHE["nc"]

    res = run_bass_kernel_spmd(nc, in_maps, core_ids=list(range(E)))
    out = np.concatenate([res.results[e]["y"] for e in range(E)], axis=0)
    return out.astype(np.float32)


if __name__ == "__main__":
    # smoke: build only
    nc = _build_bass()
    print("built ok, instructions:", len(nc.inst_map))
